# revision 5
# baseline (speedup 1.0000x reference)
"""Clustered-attention Trainium2 kernel (Bass/Tile), 8-core SPMD.

Problem (per batch b, variable k, with L=512, V=32, D=64, C=8 clusters):
    S   = sum_v key[b,:,v,:]                  # (L, D) shared key-sum
    sc  = query[b,:,k,:] @ S.T / sqrt(D)      # (L, L)
    sc  = where(label[i]==label[j], sc, -inf)
    out = softmax(sc, -1) @ value[b,:,k,:]

Sharding: 8 cores = 4 batches x 2 halves of the v axis (16 heads/core).

Design (v2, diagonal bins):
  - Host sorts each batch by label and bin-packs the 8 clusters into
    NB bins of <=128 rows (NB=5 for typical label draws), padding each
    bin to 128. No cluster straddles a bin boundary, so the score
    matrix is block-DIAGONAL over bins: only NB 128x128 blocks per
    head (vs 10 windowed chunk pairs), cutting exp and A@V work ~2.5x.
  - The cluster mask is folded into the scores matmul as 3 extra
    contraction rows encoding -beta*(l_i - l_j)^2 via
    [2b*l_j, -b*l_j^2, -b] x [l_i, 1, l_i^2]; labels are small ints so
    the terms are exact in fp32 PSUM and cancel exactly for same-
    cluster pairs. exp(z/8) is then the masked weight directly
    (beta/8 = 96 per unit label difference => underflows to 0).
  - Prologue is chunk-pipelined: key arrives per bin; each bin's
    keysum tree (DVE) + PE transpose runs as soon as its chunk lands,
    so the first scores matmul fires ~13us earlier than waiting for
    the full key.
  - Scores batch 4 heads per matmul/exp: [128, 512] tiles, one
    ACTIVATE per (bin, head-group) => 4*NB exps per core.
  - A@V accumulates E^T as lhsT with a ones-column appended to V so
    the softmax denominator lands in PSUM col 64; one reciprocal +
    one broadcast multiply per head normalize it.
  - Output fp16 in bin-padded layout, streamed per head-group.
"""

import numpy as np

import concourse.bass as bass
import concourse.tile as tile
from concourse import mybir
from concourse.masks import make_identity
from concourse.tile import TileContext, ScopedClock

B, L, V, D = 4, 512, 32, 64
NC = 8  # cores
VH = V // 2  # heads (variables) per core
NCL = 8  # clusters
BETA = 768.0  # mask weight: exp((z - BETA*diff^2)/8), BETA/8 = 96
KEXT = D + 3  # contraction rows: S + 3 mask rows
F32 = mybir.dt.float32
F16 = mybir.dt.float16
BF16 = mybir.dt.bfloat16

PROFILE = False  # set True from a harness to enable NTFF tracing
LAST_RESULT = None  # BassKernelResults of the most recent run

_PATCHED = False


def _patch_tile_drain():
    """Walrus on this image rejects multiple sync-waits on one instruction
    ("Too many sync wait commands"). Legalize by splitting surplus waits
    onto NoOp instructions inserted just before, on the same engine —
    identical semantics (the engine stalls at each wait in order)."""
    global _PATCHED
    if _PATCHED:
        return
    _PATCHED = True

    _orig_add = TileContext._add_instruction

    def _add_instruction(self, inst):
        si = getattr(inst, "sync_info", None)
        if (
            si is not None
            and si.on_wait
            and len(si.on_wait) > 1
            and inst.engine != mybir.EngineType.Unassigned
        ):
            waits = list(si.on_wait)
            for w in waits[:-1]:
                nop = mybir.InstNoOp(name=self.nc.get_next_instruction_name())
                nop.engine = inst.engine
                nop.sync_info = mybir.SyncInfo(on_wait=[w], on_update=[])
                _orig_add(self, nop)
            inst.sync_info = mybir.SyncInfo(
                on_wait=[waits[-1]], on_update=list(si.on_update or [])
            )
        _orig_add(self, inst)

    TileContext._add_instruction = _add_instruction

    def _drain_and_barrier(self, tick_clock, wait_clock):
        nc = self.nc
        drain_inst = nc.sync.drain()
        wait_clock.add_sem_waits(
            drain_inst.ins, ScopedClock({None: tick_clock.global_clock})
        )
        si = drain_inst.ins.sync_info
        if si is not None and si.on_wait and len(si.on_wait) > 1:
            waits = list(si.on_wait)
            drain_inst.ins.sync_info = mybir.SyncInfo(
                on_wait=waits[:1], on_update=list(si.on_update or [])
            )
            for i in range(1, len(waits)):
                nop = nc.sync.nop(nofuse=True, hint=f"drain_split_{i}")
                nop.ins.sync_info = mybir.SyncInfo(on_wait=[waits[i]], on_update=[])
        nc.all_engine_barrier()
        assert self.sems is not None
        popped = nc._tile_sem_poison_stack.pop()
        assert popped is self._sem_poison
        nc.clear_and_free_semaphores(list(self.sems.allocated().values()))
        nc.all_engine_barrier()

    TileContext._drain_and_barrier = _drain_and_barrier


def _build_nc(NB):
    """Build the SPMD kernel IR for NB bins (shapes depend only on NB)."""
    NP = NB * 128
    nc = bass.Bass("TRN2", target_bir_lowering=False, debug=False)

    # Host-prepared, bin-padded, label-sorted layouts (fp16/bf16):
    q_in = nc.dram_tensor("q", [KEXT, VH * NP], F16, kind="ExternalInput").ap()
    k_in = nc.dram_tensor("k", [128, NB, V, D], F16, kind="ExternalInput").ap()
    v_in = nc.dram_tensor("v", [128, NB, VH, 66], BF16, kind="ExternalInput").ap()
    sm_in = nc.dram_tensor("sm", [3, NB * 128], F16, kind="ExternalInput").ap()
    o_out = nc.dram_tensor("o", [4, 128, NB * 4 * D], F16, kind="ExternalOutput").ap()

    with TileContext(nc) as tc:
        with (
            tc.tile_pool(name="singles", bufs=1) as singles,
            tc.tile_pool(name="redpool", bufs=2) as redpool,
            tc.tile_pool(name="epool", bufs=2) as epool,
            tc.tile_pool(name="rpool", bufs=2) as rpool,
            tc.tile_pool(name="ps_s", bufs=2, space="PSUM") as ps_s,
            tc.tile_pool(name="ps_u", bufs=5, space="PSUM") as ps_u,
            tc.tile_pool(name="ps_t", bufs=1, space="PSUM") as ps_t,
        ):
            # ---- constants on Pool (gpsimd): identity for transposes,
            # junk for PE warmup, dummy to preload the exp table ----
            identity = singles.tile([128, 128], F16)
            make_identity(nc, identity)
            junk = singles.tile([128, 512], F16)
            nc.gpsimd.memset(junk, 1.0)
            dummy = singles.tile([128, 1], F32)
            nc.gpsimd.memset(dummy, 0.0)

            # ---- SBUF input tiles ----
            qv = singles.tile([KEXT, VH, NP], F16, name="qv")
            kc = [
                singles.tile([128, V * D], F16, tag=f"kc{c}", name=f"kc{c}")
                for c in range(NB)
            ]
            vcast = singles.tile([128, NB, VH, 66], BF16, name="vcast")
            # stb: [S^T rows 0..D) from keysum transposes; mask rows D..D+3)
            stb = singles.tile([KEXT, NB, 128], F16, name="stb")

            # ---- DMA issues, spread over the three DMA-capable
            # sequencers (SP, Act, Pool SWDGE), earliest-deadline
            # first. Chunk 0's key + head-group 0's q gate the first
            # exp; later chunks/groups have staggered deadlines. ----
            def key_dma(eng, c, parts):
                vstep = V // parts
                for i in range(parts):
                    eng.dma_start(
                        out=kc[c][:, i * vstep * D : (i + 1) * vstep * D],
                        in_=k_in[:, c, i * vstep : (i + 1) * vstep, :],
                    )

            def q_dma(eng, g, i, parts):
                lo = g * 4 * NP
                w = 4 * NP // parts
                eng.dma_start(
                    out=qv[:, 4 * g : 4 * (g + 1), :].rearrange("p h n -> p (h n)")[
                        :, i * w : (i + 1) * w
                    ],
                    in_=q_in[:, lo + i * w : lo + (i + 1) * w],
                )

            def v_dma(eng, c, half):
                sl = slice(half * 64, (half + 1) * 64)
                eng.dma_start(out=vcast[sl, c], in_=v_in[sl, c])

            # SP: kc0 eighths interleaved with q-g0 quarters, then
            # kc1, kc3, qg1, v1-v3, qg2, qg3, v4
            for i in range(4):
                eng = nc.sync
                eng.dma_start(
                    out=kc[0][:, i * 4 * D : (i + 1) * 4 * D],
                    in_=k_in[:, 0, i * 4 : (i + 1) * 4, :],
                )
                q_dma(nc.sync, 0, i, 4)
            key_dma(nc.sync, 1, 4)
            key_dma(nc.sync, 3, 4)
            q_dma(nc.sync, 1, 0, 2)
            q_dma(nc.sync, 1, 1, 2)
            v_dma(nc.sync, 1, 0)
            v_dma(nc.sync, 1, 1)
            v_dma(nc.sync, 2, 0)
            v_dma(nc.sync, 2, 1)
            v_dma(nc.sync, 3, 0)
            v_dma(nc.sync, 3, 1)
            q_dma(nc.sync, 2, 0, 1)
            q_dma(nc.sync, 3, 0, 1)
            v_dma(nc.sync, 4, 0)
            v_dma(nc.sync, 4, 1)

            # Act: kc0 eighths 4-7, exp-table preload, kc2, mask rows
            for i in range(4, 8):
                nc.scalar.dma_start(
                    out=kc[0][:, i * 4 * D : (i + 1) * 4 * D],
                    in_=k_in[:, 0, i * 4 : (i + 1) * 4, :],
                )
            dummyo = singles.tile([128, 1], F32)
            nc.scalar.activation(dummyo, dummy, mybir.ActivationFunctionType.Exp)
            key_dma(nc.scalar, 2, 4)
            nc.scalar.dma_start(
                out=stb[D : D + 3, :, :].rearrange("p a b -> p (a b)"), in_=sm_in
            )

            # Pool (SWDGE): kc4, v0
            key_dma(nc.gpsimd, 4, 4)
            v_dma(nc.gpsimd, 0, 0)
            v_dma(nc.gpsimd, 0, 1)

            # ---- PE warmup so the HAM clock ramps before the pipeline;
            # bridges the gap until the first transpose/scores. ----
            for w in range(14):
                wps = ps_s.tile([128, 4, 128], F32, tag="ps", name=f"warm{w}")
                nc.tensor.matmul(
                    wps.rearrange("p a b -> p (a b)"), lhsT=identity, rhs=junk,
                    start=True, stop=True,
                )

            # ---- keysum per chunk: tree adds on DVE as each chunk
            # lands, then PE transpose into stb rows 0..D ----
            def make_stb(c):
                t1a = redpool.tile([128, 512], F16, tag="t1a", name=f"t1a{c}")
                nc.vector.tensor_tensor(
                    out=t1a, in0=kc[c][:, 0:512], in1=kc[c][:, 512:1024],
                    op=mybir.AluOpType.add,
                )
                t1b = redpool.tile([128, 512], F16, tag="t1b", name=f"t1b{c}")
                nc.vector.tensor_tensor(
                    out=t1b, in0=kc[c][:, 1024:1536], in1=kc[c][:, 1536:2048],
                    op=mybir.AluOpType.add,
                )
                cur = redpool.tile([128, 512], F16, tag="t2", name=f"t2{c}")
                nc.vector.tensor_tensor(out=cur, in0=t1a, in1=t1b,
                                        op=mybir.AluOpType.add)
                w = 512
                while w > D:
                    w //= 2
                    nxt = redpool.tile([128, w], F16, tag=f"r{w}", name=f"r{w}_{c}")
                    nc.vector.tensor_tensor(
                        out=nxt, in0=cur[:, 0:w], in1=cur[:, w : 2 * w],
                        op=mybir.AluOpType.add,
                    )
                    cur = nxt
                st_ps = ps_t.tile([D, 128], F16, tag="st", name=f"st{c}")
                nc.tensor.transpose(st_ps, cur, identity)
                nc.vector.tensor_copy(stb[0:D, c, :], st_ps)

            for c in range(NB):
                make_stb(c)

            # ---- main pipeline: 4 head-groups of 4 heads; scores+exp
            # per (group, bin); A@V of the previous group interleaved
            # between score matmuls so the PE never idles on exp ----
            e_tiles = [[None] * NB for _ in range(4)]
            psu = {}
            oc = [
                singles.tile([128, NB, 4, D], F16, tag=f"oc{g}", name=f"oc{g}")
                for g in range(4)
            ]

            def scores(g, c):
                ps = ps_s.tile([128, 4, 128], F32, tag="ps", name=f"ps{g}_{c}")
                nc.tensor.matmul(
                    ps, lhsT=stb[:, c, :],
                    rhs=qv[0:KEXT, 4 * g : 4 * (g + 1), c * 128 : (c + 1) * 128],
                    start=True, stop=True,
                )
                e = epool.tile([128, 4, 128], BF16, tag=f"e{c}", name=f"e{g}_{c}")
                nc.scalar.activation(
                    e, ps, mybir.ActivationFunctionType.Exp, scale=0.125
                )
                e_tiles[g][c] = e

            def av(g, c):
                for hh in range(4):
                    if c == 0:
                        psu[(g, hh)] = ps_u.tile(
                            [128, NB, 65], F32, tag="psu", name=f"psu{g}_{hh}"
                        )
                    nc.tensor.matmul(
                        psu[(g, hh)][:, c, :],
                        lhsT=e_tiles[g][c][:, hh, :],
                        rhs=vcast[:, c, 4 * g + hh, 0:65],
                        start=True, stop=True,
                    )

            def norm_out(g):
                for hh in range(4):
                    p = psu[(g, hh)]
                    rinv = rpool.tile([128, NB], F32, tag="rinv",
                                      name=f"rinv{g}_{hh}")
                    nc.vector.reciprocal(rinv, p[:, :, D])
                    rinv_b = bass.AP(
                        tensor=rinv.tensor, offset=rinv.offset,
                        ap=[list(rinv.ap[0]), list(rinv.ap[1]), [0, D]],
                    )
                    nc.vector.tensor_tensor(
                        out=oc[g][:, :, hh, :], in0=p[:, :, 0:D], in1=rinv_b,
                        op=mybir.AluOpType.mult,
                    )
                flat = oc[g].rearrange("p a b d -> p (a b d)")
                parts = 8 if g == 3 else 4
                step = 128 // parts
                for i in range(parts):
                    sl = slice(i * step, (i + 1) * step)
                    nc.sync.dma_start(out=o_out[g, sl], in_=flat[sl])

            for c in range(NB):
                scores(0, c)
            for g in range(1, 4):
                for c in range(NB):
                    scores(g, c)
                    av(g - 1, c)
                norm_out(g - 1)
            for c in range(NB):
                av(3, c)
            norm_out(3)
    return nc


_NC_CACHE = {}


def _get_nc(NB):
    if NB not in _NC_CACHE:
        _patch_tile_drain()
        _NC_CACHE[NB] = _build_nc(NB)
    return _NC_CACHE[NB]


def _plan_bins(sizes):
    """First-fit-decreasing pack of cluster sizes into bins of cap 128."""
    order = np.argsort(-sizes, kind="stable")
    bins, loads = [], []
    for cid in order:
        s = int(sizes[cid])
        for t in range(len(bins)):
            if loads[t] + s <= 128:
                bins[t].append(int(cid))
                loads[t] += s
                break
        else:
            bins.append([int(cid)])
            loads.append(s)
    return bins


def kernel(query, key, value, label_arr):
    """Full inputs (B,L,V,D)/(B,L) -> full output (B,L,V,D)."""
    global LAST_RESULT
    import ml_dtypes
    from concourse.bass_utils import run_bass_kernel_spmd

    query = np.asarray(query, dtype=np.float32)
    key = np.asarray(key, dtype=np.float32)
    value = np.asarray(value, dtype=np.float32)
    labels = np.asarray(label_arr).astype(np.int64)

    # Per-batch bin plan (shared by the 2 cores of each batch).
    plans = []
    for b in range(B):
        sizes = np.bincount(labels[b], minlength=NCL)
        bins = _plan_bins(sizes)
        order = np.argsort(labels[b], kind="stable")
        starts = np.zeros(NCL + 1, dtype=np.int64)
        np.cumsum(sizes, out=starts[1:])
        rows = []  # original position per padded row, -1 = pad
        for t, cl in enumerate(bins):
            r = []
            for cid in cl:
                r.extend(order[starts[cid] : starts[cid + 1]].tolist())
            r.extend([-1] * (128 - len(r)))
            rows.append(r)
        plans.append(rows)
    NB = max(len(p) for p in plans)
    for p in plans:
        while len(p) < NB:
            p.append([-1] * 128)
    NP = NB * 128

    in_maps = []
    rowmaps = []
    for c in range(NC):
        b, v0 = c // 2, (c % 2) * VH
        rowmap = np.array(plans[b], dtype=np.int64).reshape(NP)  # [NP]
        rowmaps.append(rowmap)
        valid = rowmap >= 0
        pos = rowmap[valid]
        labp = np.full(NP, NCL, dtype=np.float32)
        labp[valid] = labels[b][pos]

        qp = np.zeros((KEXT, VH, NP), dtype=np.float16)
        qp[0:D, :, valid] = query[b][pos][:, v0 : v0 + VH, :].transpose(2, 1, 0)
        qp[D] = labp[None, :]
        qp[D + 1] = 1.0
        qp[D + 2] = labp[None, :] ** 2

        kp = np.zeros((NP, V, D), dtype=np.float16)
        kp[valid] = key[b][pos]

        vp = np.zeros((NP, VH, 66), dtype=ml_dtypes.bfloat16)
        vp[valid, :, 0:D] = value[b][pos][:, v0 : v0 + VH, :].astype(
            ml_dtypes.bfloat16
        )
        vp[valid, :, D] = 1.0

        sm = np.empty((3, NP), dtype=np.float16)
        sm[0] = 2.0 * BETA * labp
        sm[1] = -BETA * labp**2
        sm[2] = -BETA

        in_maps.append({
            "q": np.ascontiguousarray(qp.reshape(KEXT, VH * NP)),
            "k": np.ascontiguousarray(
                kp.reshape(NB, 128, V, D).transpose(1, 0, 2, 3)
            ),
            "v": np.ascontiguousarray(
                vp.reshape(NB, 128, VH, 66).transpose(1, 0, 2, 3)
            ),
            "sm": sm,
        })

    nc = _get_nc(NB)
    kwargs = {}
    if PROFILE:
        kwargs["trace"] = True
    res = run_bass_kernel_spmd(nc, in_maps, list(range(NC)), **kwargs)
    LAST_RESULT = res

    out = np.empty((B, L, V, D), dtype=np.float32)
    for c in range(NC):
        b, v0 = c // 2, (c % 2) * VH
        rowmap = rowmaps[c]
        valid = rowmap >= 0
        pos = rowmap[valid]
        # o: [g, p, (t hh d)] -> padded rows [NP, 4g+hh, d]
        o = res.results[c]["o"].astype(np.float32)
        o = o.reshape(4, 128, NB, 4, D).transpose(2, 1, 0, 3, 4).reshape(NP, VH, D)
        out[b][pos, v0 : v0 + VH, :] = o[valid]
    return out


# revision 6
# speedup vs baseline: 1.0215x; 1.0215x over previous
"""Clustered-attention Trainium2 kernel (Bass/Tile), 8-core SPMD.

Problem (per batch b, variable k, with L=512, V=32, D=64, C=8 clusters):
    S   = sum_v key[b,:,v,:]                  # (L, D) shared key-sum
    sc  = query[b,:,k,:] @ S.T / sqrt(D)      # (L, L)
    sc  = where(label[i]==label[j], sc, -inf)
    out = softmax(sc, -1) @ value[b,:,k,:]

Sharding: 8 cores = 4 batches x 2 halves of the v axis (16 heads/core).

Design (v2, diagonal bins):
  - Host sorts each batch by label and bin-packs the 8 clusters into
    NB bins of <=128 rows (NB=5 for typical label draws), padding each
    bin to 128. No cluster straddles a bin boundary, so the score
    matrix is block-DIAGONAL over bins: only NB 128x128 blocks per
    head (vs 10 windowed chunk pairs), cutting exp and A@V work ~2.5x.
  - The cluster mask is folded into the scores matmul as 3 extra
    contraction rows encoding -beta*(l_i - l_j)^2 via
    [2b*l_j, -b*l_j^2, -b] x [l_i, 1, l_i^2]; labels are small ints so
    the terms are exact in fp32 PSUM and cancel exactly for same-
    cluster pairs. exp(z/8) is then the masked weight directly
    (beta/8 = 96 per unit label difference => underflows to 0).
  - Prologue is chunk-pipelined: key arrives per bin; each bin's
    keysum tree (DVE) + PE transpose runs as soon as its chunk lands,
    so the first scores matmul fires ~13us earlier than waiting for
    the full key.
  - Scores batch 4 heads per matmul/exp: [128, 512] tiles, one
    ACTIVATE per (bin, head-group) => 4*NB exps per core.
  - A@V accumulates E^T as lhsT with a ones-column appended to V so
    the softmax denominator lands in PSUM col 64; one reciprocal +
    one broadcast multiply per head normalize it.
  - Output fp16 in bin-padded layout, streamed per head-group.
"""

import numpy as np

import concourse.bass as bass
import concourse.tile as tile
from concourse import mybir
from concourse.masks import make_identity
from concourse.tile import TileContext, ScopedClock

B, L, V, D = 4, 512, 32, 64
NC = 8  # cores
VH = V // 2  # heads (variables) per core
NCL = 8  # clusters
BETA = 768.0  # mask weight: exp((z - BETA*diff^2)/8), BETA/8 = 96
KEXT = D + 3  # contraction rows: S + 3 mask rows
F32 = mybir.dt.float32
F16 = mybir.dt.float16
BF16 = mybir.dt.bfloat16

PROFILE = False  # set True from a harness to enable NTFF tracing
LAST_RESULT = None  # BassKernelResults of the most recent run

_PATCHED = False


def _patch_tile_drain():
    """Walrus on this image rejects multiple sync-waits on one instruction
    ("Too many sync wait commands"). Legalize by splitting surplus waits
    onto NoOp instructions inserted just before, on the same engine —
    identical semantics (the engine stalls at each wait in order)."""
    global _PATCHED
    if _PATCHED:
        return
    _PATCHED = True

    _orig_add = TileContext._add_instruction

    def _add_instruction(self, inst):
        si = getattr(inst, "sync_info", None)
        if (
            si is not None
            and si.on_wait
            and len(si.on_wait) > 1
            and inst.engine != mybir.EngineType.Unassigned
        ):
            waits = list(si.on_wait)
            for w in waits[:-1]:
                nop = mybir.InstNoOp(name=self.nc.get_next_instruction_name())
                nop.engine = inst.engine
                nop.sync_info = mybir.SyncInfo(on_wait=[w], on_update=[])
                _orig_add(self, nop)
            inst.sync_info = mybir.SyncInfo(
                on_wait=[waits[-1]], on_update=list(si.on_update or [])
            )
        _orig_add(self, inst)

    TileContext._add_instruction = _add_instruction

    def _drain_and_barrier(self, tick_clock, wait_clock):
        nc = self.nc
        drain_inst = nc.sync.drain()
        wait_clock.add_sem_waits(
            drain_inst.ins, ScopedClock({None: tick_clock.global_clock})
        )
        si = drain_inst.ins.sync_info
        if si is not None and si.on_wait and len(si.on_wait) > 1:
            waits = list(si.on_wait)
            drain_inst.ins.sync_info = mybir.SyncInfo(
                on_wait=waits[:1], on_update=list(si.on_update or [])
            )
            for i in range(1, len(waits)):
                nop = nc.sync.nop(nofuse=True, hint=f"drain_split_{i}")
                nop.ins.sync_info = mybir.SyncInfo(on_wait=[waits[i]], on_update=[])
        nc.all_engine_barrier()
        assert self.sems is not None
        popped = nc._tile_sem_poison_stack.pop()
        assert popped is self._sem_poison
        nc.clear_and_free_semaphores(list(self.sems.allocated().values()))
        nc.all_engine_barrier()

    TileContext._drain_and_barrier = _drain_and_barrier


def _build_nc(NB):
    """Build the SPMD kernel IR for NB bins (shapes depend only on NB)."""
    NP = NB * 128
    nc = bass.Bass("TRN2", target_bir_lowering=False, debug=False)

    # Host-prepared, bin-padded, label-sorted layouts (fp16/bf16):
    q_in = nc.dram_tensor("q", [KEXT, VH * NP], F16, kind="ExternalInput").ap()
    k_in = nc.dram_tensor("k", [128, NB, V, D], F16, kind="ExternalInput").ap()
    v_in = nc.dram_tensor("v", [128, NB, VH, 66], BF16, kind="ExternalInput").ap()
    sm_in = nc.dram_tensor("sm", [3, NB * 128], F16, kind="ExternalInput").ap()
    o_out = nc.dram_tensor("o", [4, 128, NB * 4 * D], F16, kind="ExternalOutput").ap()

    with TileContext(nc) as tc:
        with (
            tc.tile_pool(name="singles", bufs=1) as singles,
            tc.tile_pool(name="redpool", bufs=2) as redpool,
            tc.tile_pool(name="epool", bufs=2) as epool,
            tc.tile_pool(name="rpool", bufs=2) as rpool,
            tc.tile_pool(name="ps_s", bufs=2, space="PSUM") as ps_s,
            tc.tile_pool(name="ps_u", bufs=5, space="PSUM") as ps_u,
            tc.tile_pool(name="ps_t", bufs=1, space="PSUM") as ps_t,
        ):
            # ---- constants on Pool (gpsimd): identity for transposes,
            # junk for PE warmup, dummy to preload the exp table ----
            identity = singles.tile([128, 128], F16)
            make_identity(nc, identity)
            junk = singles.tile([128, 512], F16)
            nc.gpsimd.memset(junk, 1.0)
            dummy = singles.tile([128, 1], F32)
            nc.gpsimd.memset(dummy, 0.0)

            # ---- SBUF input tiles ----
            qv = singles.tile([KEXT, VH, NP], F16, name="qv")
            kc = [
                singles.tile([128, V * D], F16, tag=f"kc{c}", name=f"kc{c}")
                for c in range(NB)
            ]
            vcast = singles.tile([128, NB, VH, 66], BF16, name="vcast")
            # stb: [S^T rows 0..D) from keysum transposes; mask rows D..D+3)
            stb = singles.tile([KEXT, NB, 128], F16, name="stb")

            # ---- DMA issues. Descriptors stripe across all 16 DMA
            # engines (~180ns/descriptor up to ~4KB), so ONE fat DMA
            # per tensor region beats many thin splits: key chunk =
            # 128 x 4KB descriptors (~1.4us), all of v = 128 x 10.5KB,
            # one q-group = 70 x 5KB. Three rings: SP, Act, Pool. ----
            def q_dma(eng, g):
                eng.dma_start(
                    out=qv[:, 4 * g : 4 * (g + 1), :].rearrange("p h n -> p (h n)"),
                    in_=q_in[:, g * 4 * NP : (g + 1) * 4 * NP],
                )

            # SP ring: key chunks 0-3 (chunk c gates bin c's tree/stb)
            for c in range(4):
                nc.sync.dma_start(out=kc[c], in_=k_in[:, c])
            # Act ring: last key chunk, q-g0, mask rows, exp preload,
            # then the remaining q groups
            if NB > 4:
                for c in range(4, NB):
                    nc.scalar.dma_start(out=kc[c], in_=k_in[:, c])
            q_dma(nc.scalar, 0)
            nc.scalar.dma_start(
                out=stb[D : D + 3, :, :].rearrange("p a b -> p (a b)"), in_=sm_in
            )
            dummyo = singles.tile([128, 1], F32)
            nc.scalar.activation(dummyo, dummy, mybir.ActivationFunctionType.Exp)
            q_dma(nc.scalar, 1)
            q_dma(nc.scalar, 2)
            q_dma(nc.scalar, 3)
            # Pool ring (SWDGE): all of v in one fat DMA
            nc.gpsimd.dma_start(out=vcast, in_=v_in)

            # ---- PE warmup so the HAM clock ramps before the pipeline;
            # bridges the gap until the first transpose/scores. ----
            for w in range(14):
                wps = ps_s.tile([128, 4, 128], F32, tag="ps", name=f"warm{w}")
                nc.tensor.matmul(
                    wps.rearrange("p a b -> p (a b)"), lhsT=identity, rhs=junk,
                    start=True, stop=True,
                )

            # ---- keysum per chunk: tree adds on DVE as each chunk
            # lands, then PE transpose into stb rows 0..D ----
            def make_stb(c):
                t1a = redpool.tile([128, 512], F16, tag="t1a", name=f"t1a{c}")
                nc.vector.tensor_tensor(
                    out=t1a, in0=kc[c][:, 0:512], in1=kc[c][:, 512:1024],
                    op=mybir.AluOpType.add,
                )
                t1b = redpool.tile([128, 512], F16, tag="t1b", name=f"t1b{c}")
                nc.vector.tensor_tensor(
                    out=t1b, in0=kc[c][:, 1024:1536], in1=kc[c][:, 1536:2048],
                    op=mybir.AluOpType.add,
                )
                cur = redpool.tile([128, 512], F16, tag="t2", name=f"t2{c}")
                nc.vector.tensor_tensor(out=cur, in0=t1a, in1=t1b,
                                        op=mybir.AluOpType.add)
                w = 512
                while w > D:
                    w //= 2
                    nxt = redpool.tile([128, w], F16, tag=f"r{w}", name=f"r{w}_{c}")
                    nc.vector.tensor_tensor(
                        out=nxt, in0=cur[:, 0:w], in1=cur[:, w : 2 * w],
                        op=mybir.AluOpType.add,
                    )
                    cur = nxt
                st_ps = ps_t.tile([D, 128], F16, tag="st", name=f"st{c}")
                nc.tensor.transpose(st_ps, cur, identity)
                nc.vector.tensor_copy(stb[0:D, c, :], st_ps)

            for c in range(NB):
                make_stb(c)

            # ---- main pipeline: 4 head-groups of 4 heads; scores+exp
            # per (group, bin); A@V of the previous group interleaved
            # between score matmuls so the PE never idles on exp ----
            e_tiles = [[None] * NB for _ in range(4)]
            psu = {}
            oc = [
                singles.tile([128, NB, 4, D], F16, tag=f"oc{g}", name=f"oc{g}")
                for g in range(4)
            ]

            def scores(g, c):
                ps = ps_s.tile([128, 4, 128], F32, tag="ps", name=f"ps{g}_{c}")
                nc.tensor.matmul(
                    ps, lhsT=stb[:, c, :],
                    rhs=qv[0:KEXT, 4 * g : 4 * (g + 1), c * 128 : (c + 1) * 128],
                    start=True, stop=True,
                )
                e = epool.tile([128, 4, 128], BF16, tag=f"e{c}", name=f"e{g}_{c}")
                nc.scalar.activation(
                    e, ps, mybir.ActivationFunctionType.Exp, scale=0.125
                )
                e_tiles[g][c] = e

            def av(g, c):
                for hh in range(4):
                    if c == 0:
                        psu[(g, hh)] = ps_u.tile(
                            [128, NB, 65], F32, tag="psu", name=f"psu{g}_{hh}"
                        )
                    nc.tensor.matmul(
                        psu[(g, hh)][:, c, :],
                        lhsT=e_tiles[g][c][:, hh, :],
                        rhs=vcast[:, c, 4 * g + hh, 0:65],
                        start=True, stop=True,
                    )

            def norm_out(g):
                for hh in range(4):
                    p = psu[(g, hh)]
                    rinv = rpool.tile([128, NB], F32, tag="rinv",
                                      name=f"rinv{g}_{hh}")
                    nc.vector.reciprocal(rinv, p[:, :, D])
                    rinv_b = bass.AP(
                        tensor=rinv.tensor, offset=rinv.offset,
                        ap=[list(rinv.ap[0]), list(rinv.ap[1]), [0, D]],
                    )
                    nc.vector.tensor_tensor(
                        out=oc[g][:, :, hh, :], in0=p[:, :, 0:D], in1=rinv_b,
                        op=mybir.AluOpType.mult,
                    )
                flat = oc[g].rearrange("p a b d -> p (a b d)")
                parts = 8 if g == 3 else 4
                step = 128 // parts
                for i in range(parts):
                    sl = slice(i * step, (i + 1) * step)
                    nc.sync.dma_start(out=o_out[g, sl], in_=flat[sl])

            for c in range(NB):
                scores(0, c)
            for g in range(1, 4):
                for c in range(NB):
                    scores(g, c)
                    av(g - 1, c)
                norm_out(g - 1)
            for c in range(NB):
                av(3, c)
            norm_out(3)
    return nc


_NC_CACHE = {}


def _get_nc(NB):
    if NB not in _NC_CACHE:
        _patch_tile_drain()
        _NC_CACHE[NB] = _build_nc(NB)
    return _NC_CACHE[NB]


def _plan_bins(sizes):
    """First-fit-decreasing pack of cluster sizes into bins of cap 128."""
    order = np.argsort(-sizes, kind="stable")
    bins, loads = [], []
    for cid in order:
        s = int(sizes[cid])
        for t in range(len(bins)):
            if loads[t] + s <= 128:
                bins[t].append(int(cid))
                loads[t] += s
                break
        else:
            bins.append([int(cid)])
            loads.append(s)
    return bins


def kernel(query, key, value, label_arr):
    """Full inputs (B,L,V,D)/(B,L) -> full output (B,L,V,D)."""
    global LAST_RESULT
    import ml_dtypes
    from concourse.bass_utils import run_bass_kernel_spmd

    query = np.asarray(query, dtype=np.float32)
    key = np.asarray(key, dtype=np.float32)
    value = np.asarray(value, dtype=np.float32)
    labels = np.asarray(label_arr).astype(np.int64)

    # Per-batch bin plan (shared by the 2 cores of each batch).
    plans = []
    for b in range(B):
        sizes = np.bincount(labels[b], minlength=NCL)
        bins = _plan_bins(sizes)
        order = np.argsort(labels[b], kind="stable")
        starts = np.zeros(NCL + 1, dtype=np.int64)
        np.cumsum(sizes, out=starts[1:])
        rows = []  # original position per padded row, -1 = pad
        for t, cl in enumerate(bins):
            r = []
            for cid in cl:
                r.extend(order[starts[cid] : starts[cid + 1]].tolist())
            r.extend([-1] * (128 - len(r)))
            rows.append(r)
        plans.append(rows)
    NB = max(len(p) for p in plans)
    for p in plans:
        while len(p) < NB:
            p.append([-1] * 128)
    NP = NB * 128

    in_maps = []
    rowmaps = []
    for c in range(NC):
        b, v0 = c // 2, (c % 2) * VH
        rowmap = np.array(plans[b], dtype=np.int64).reshape(NP)  # [NP]
        rowmaps.append(rowmap)
        valid = rowmap >= 0
        pos = rowmap[valid]
        labp = np.full(NP, NCL, dtype=np.float32)
        labp[valid] = labels[b][pos]

        qp = np.zeros((KEXT, VH, NP), dtype=np.float16)
        qp[0:D, :, valid] = query[b][pos][:, v0 : v0 + VH, :].transpose(2, 1, 0)
        qp[D] = labp[None, :]
        qp[D + 1] = 1.0
        qp[D + 2] = labp[None, :] ** 2

        kp = np.zeros((NP, V, D), dtype=np.float16)
        kp[valid] = key[b][pos]

        vp = np.zeros((NP, VH, 66), dtype=ml_dtypes.bfloat16)
        vp[valid, :, 0:D] = value[b][pos][:, v0 : v0 + VH, :].astype(
            ml_dtypes.bfloat16
        )
        vp[valid, :, D] = 1.0

        sm = np.empty((3, NP), dtype=np.float16)
        sm[0] = 2.0 * BETA * labp
        sm[1] = -BETA * labp**2
        sm[2] = -BETA

        in_maps.append({
            "q": np.ascontiguousarray(qp.reshape(KEXT, VH * NP)),
            "k": np.ascontiguousarray(
                kp.reshape(NB, 128, V, D).transpose(1, 0, 2, 3)
            ),
            "v": np.ascontiguousarray(
                vp.reshape(NB, 128, VH, 66).transpose(1, 0, 2, 3)
            ),
            "sm": sm,
        })

    nc = _get_nc(NB)
    kwargs = {}
    if PROFILE:
        kwargs["trace"] = True
    res = run_bass_kernel_spmd(nc, in_maps, list(range(NC)), **kwargs)
    LAST_RESULT = res

    out = np.empty((B, L, V, D), dtype=np.float32)
    for c in range(NC):
        b, v0 = c // 2, (c % 2) * VH
        rowmap = rowmaps[c]
        valid = rowmap >= 0
        pos = rowmap[valid]
        # o: [g, p, (t hh d)] -> padded rows [NP, 4g+hh, d]
        o = res.results[c]["o"].astype(np.float32)
        o = o.reshape(4, 128, NB, 4, D).transpose(2, 1, 0, 3, 4).reshape(NP, VH, D)
        out[b][pos, v0 : v0 + VH, :] = o[valid]
    return out
